# revision 29
# baseline (speedup 1.0000x reference)
"""Trainium2 Bass kernel for a biased self-attention block (fp8 DoubleRow).

Reference computation (per sample b):
    hn = GroupNorm32(x) * gamma + beta
    q/k/v = 1x1 conv (C x C matmul) of hn
    s = q^T k / sqrt(C)            [hw_q, hw_k]
    attn = softmax(s) * mask; attn /= sum(attn)   (== exp(s)*m / sum(exp(s)*m))
    out = v @ attn^T; y = x + Wo out + bo

Sharding: 8 cores = 4 samples x 2 query-halves. Each core receives its
sample's full x (spatially rotated so its query half occupies positions
0..2047), computes GroupNorm + K/V for all masked-in key positions and
Q/attention output for its 2048 queries only.

All matmuls run in fp8 e4m3 with perf_mode=DoubleRow (two contraction
slices of 128 per instruction, ~1.5x bf16 throughput).  Operands are laid
out as [128, n_ktiles, free] so a DoubleRow matmul consumes slices
[:, 2i:2i+2, :].  Weights are scaled x16 host-side so their values sit in
the fp8 normal range; the x16 is compensated at each PSUM->SBUF activation
(scale=1/16).  Attention probabilities are computed as exp(s/sqrt(C) - 2)
(the renormalization cancels any constant shift; -2 keeps exp below the
TRN e4m3 max of 240).  The attention output (magnitude ~0.04) is scaled
x16 into fp8 via dinv = 16/denominator; the final activation applies
1/256 (16 weight x 16 outn) plus the folded bias before the residual add.

Scores are computed transposed ([key, query] = k_mat^T q) so softmax's key
dimension lands on PSUM partitions; the 0/1 mask becomes a per-partition
additive log-mask inside the Exp activation, and the masked row-sum
(denominator) is one extra DoubleRow matmul with an all-ones stationary.
"""

import sys

sys.path.insert(0, "/opt/trn_rl_repo")

import numpy as np
import ml_dtypes

import concourse.bass as bass
import concourse.tile as tile
from concourse import bacc, mybir
from concourse.bass_utils import run_bass_kernel_spmd

F32 = mybir.dt.float32
BF16 = mybir.dt.bfloat16
FP8 = mybir.dt.float8e4
AX = mybir.AxisListType
ALU = mybir.AluOpType
ACTF = mybir.ActivationFunctionType
DR = mybir.MatmulPerfMode.DoubleRow

B, C, HGT, WID = 4, 512, 64, 64
HW = HGT * WID          # 4096
GROUPS = 32
GSIZE = C // GROUPS     # 16 channels per group
EPS = 1e-6
NCH = C // 128          # 4 channel chunks
NQ = HW // 2            # 2048 queries per core
QT = 512                # query tile (matmul free dim)
NQT = NQ // QT          # 4 query tiles
NW = NQ // 512          # 4 spatial windows of 512 for q projection
NKM = 2176              # compacted (masked-in) key capacity, 17 windows of 128
NEG = -30000.0          # log(0) stand-in for the additive mask
SHIFT = 2.0             # constant subtracted inside exp (renorm cancels it)
WS = 16.0               # host-side weight scale into fp8 normal range


def build_program(loop_n: int = 1, loop_phase: str = "all", nkm: int = NKM):
    NKWM = nkm // 128   # key windows after mask compaction (17 default)
    nc = bacc.Bacc()
    xb_d = nc.declare_dram_parameter("xbf", [C, HW], BF16, isOutput=False)
    xq_d = nc.declare_dram_parameter("xq", [C, NQ], F32, isOutput=False)
    lm_d = nc.declare_dram_parameter("lmask", [nkm], F32, isOutput=False)
    xm_d = nc.declare_dram_parameter("xm", [C, nkm], BF16, isOutput=False)
    gam_d = nc.declare_dram_parameter("gamma", [C], F32, isOutput=False)
    bet_d = nc.declare_dram_parameter("beta", [C], F32, isOutput=False)
    wq_d = nc.declare_dram_parameter("wqd", [128, NCH, C], FP8, isOutput=False)
    wk_d = nc.declare_dram_parameter("wkd", [128, NCH, C], FP8, isOutput=False)
    wv_d = nc.declare_dram_parameter("wvd", [128, NCH, C], FP8, isOutput=False)
    wo_d = nc.declare_dram_parameter("wod", [128, NCH, C], FP8, isOutput=False)
    bq_d = nc.declare_dram_parameter("bq", [C], F32, isOutput=False)
    bk_d = nc.declare_dram_parameter("bk", [C], F32, isOutput=False)
    ind_d = nc.declare_dram_parameter("ind", [8, 128], F32, isOutput=False)
    ind2_d = nc.declare_dram_parameter("ind2", [128, 8], F32, isOutput=False)
    y_d = nc.declare_dram_parameter("y", [C, NQ], F32, isOutput=True)

    qscale = 1.0 / np.sqrt(C)

    with tile.TileContext(nc) as tc:
        with tc.tile_pool(name="persist", bufs=1) as pp:
            # --- weights / constants resident in SBUF ---
            wq_sb = pp.tile([128, NCH, C], FP8, tag="wq")
            wk_sb = pp.tile([128, NCH, C], FP8, tag="wk")
            wv_sb = pp.tile([128, NCH, C], FP8, tag="wv")
            wo_sb = pp.tile([128, NCH, C], FP8, tag="wo")

            # per-chunk vectors packed as [128, NCH] (column = chunk)
            gam_sb = pp.tile([128, NCH], F32, tag="gam")
            bet_sb = pp.tile([128, NCH], F32, tag="bet")
            bq_sb = pp.tile([128, NCH], F32, tag="bq")
            bk_sb = pp.tile([128, NCH], F32, tag="bk")
            lm_sb = pp.tile([128, NKWM], F32, tag="lmask")
            ones_sb = pp.tile([128, 128], BF16, tag="ones")
            ind_sb = pp.tile([8, 128], F32, tag="ind")
            ind2_sb = pp.tile([128, 8], F32, tag="ind2")

            # --- persistent activations (DoubleRow layouts) ---
            k_sb = pp.tile([128, NCH, nkm], FP8, tag="kd")     # [c%128, c//128, key]
            q_sb = pp.tile([128, NCH, NQ], FP8, tag="qd")      # [c%128, c//128, q]
            vt_sb = pp.tile([128, NKWM, C], FP8, tag="vtd")    # [key%128, kw, c]

            # ================= phase 1: groupnorm stats + projections ========
            import contextlib

            loop_ctx = contextlib.ExitStack()
            if loop_n > 1 and loop_phase in ("all", "ph1"):
                loop_ctx.enter_context(tc.For_i(0, loop_n, 1))
            with (
                tc.tile_pool(name="ph1", bufs=1) as p1,
                tc.tile_pool(name="ph1psum", bufs=1, space="PSUM") as p1p,
            ):
                # Three DMA lanes (SP HWDGE, ACT HWDGE, gpsimd SWDGE), each
                # ordered by when phase 1 needs the data.
                xf = [p1.tile([128, HW], BF16, name="xf", tag=f"x{i}") for i in range(NCH)]
                HHW = HW // 2

                def xdma(eng, i, piece):
                    sl = slice(piece * HHW, (piece + 1) * HHW)
                    eng.dma_start(out=xf[i][:, sl], in_=xb_d[bass.ts(i, 128), sl])

                def vdma(eng, v_d, t):
                    eng.dma_start(out=t, in_=v_d[:].rearrange("(i p) -> p i", p=128))

                # ACT lane: first halves of chunks 0/1, earliest weights, then
                # the tiny constants the stats chains need.
                xdma(nc.scalar, 0, 1)
                xdma(nc.scalar, 1, 1)
                vdma(nc.scalar, gam_d, gam_sb)
                vdma(nc.scalar, bet_d, bet_sb)
                nc.scalar.dma_start(out=ind_sb, in_=ind_d[:, :])
                nc.scalar.dma_start(out=ind2_sb, in_=ind2_d[:, :])
                nc.scalar.dma_start(out=wk_sb, in_=wk_d[:, :, :])
                nc.scalar.dma_start(out=wq_sb, in_=wq_d[:, :, :])
                # SP lane
                xdma(nc.sync, 0, 0)
                xdma(nc.sync, 1, 0)
                xdma(nc.sync, 2, 0)
                xdma(nc.sync, 3, 0)
                vdma(nc.sync, bk_d, bk_sb)
                vdma(nc.sync, bq_d, bq_sb)
                nc.sync.dma_start(out=lm_sb, in_=lm_d[:].rearrange("(w p) -> p w", p=128))
                nc.sync.dma_start(out=wv_sb, in_=wv_d[:, :, :])
                xm_sb = [
                    p1.tile([128, nkm], BF16, name="xm_sb", tag=f"xm{i}")
                    for i in range(NCH)
                ]
                # gpsimd (SWDGE) lane
                xdma(nc.gpsimd, 2, 1)
                xdma(nc.gpsimd, 3, 1)
                for i in range(NCH):
                    nc.gpsimd.dma_start(out=xm_sb[i], in_=xm_d[bass.ts(i, 128), :])
                nc.gpsimd.dma_start(out=wo_sb, in_=wo_d[:, :, :])
                nc.vector.memset(ones_sb, 1.0)

                # Per-chunk stats chains: chunk c's normalization params are
                # ready right after ITS bn_stats, so PE projections can begin
                # after chunk 0 instead of after all four chunks.
                eps_sb = p1.tile([8, 1], F32, tag="eps")
                nc.vector.memset(eps_sb, EPS)
                scale4 = p1.tile([128, NCH], F32, tag="scale4")
                shift4 = p1.tile([128, NCH], F32, tag="shift4")
                scale_sb = [scale4[:, i : i + 1] for i in range(NCH)]
                shift_sb = [shift4[:, i : i + 1] for i in range(NCH)]

                def bn_chunk(i):
                    xr = xf[i].rearrange("p (n f) -> p n f", f=512)
                    st = p1.tile([128, 8, 6], F32, name="bnst", tag="bnst", bufs=2)
                    for sg in range(8):
                        nc.vector.bn_stats(out=st[:, sg, :], in_=xr[:, sg, :])
                    st2 = p1.tile([128, 3], F32, name="st2", tag=f"st2_{i}")
                    nc.vector.bn_aggr(out=st2[:, 0:2], in_=st)
                    return st2

                def chain_chunk(i, st2):
                    # st2 cols: mean, var, mean^2 (per channel)
                    nc.scalar.activation(out=st2[:, 2:3], in_=st2[:, 0:1], func=ACTF.Square)
                    z_ps = p1p.tile([8, 3], F32, name="z_ps", tag="mr", bufs=2)
                    nc.tensor.matmul(z_ps, ind2_sb, st2, start=True, stop=True)
                    z_sb = p1.tile([8, 3], F32, name="z_sb", tag=f"z_sb{i}")
                    nc.scalar.copy(out=z_sb, in_=z_ps)
                    stat2 = p1.tile([8, 2], F32, name="stat2", tag=f"stat2_{i}")
                    nc.scalar.mul(out=stat2[:, 0:1], in_=z_sb[:, 0:1], mul=1.0 / GSIZE)
                    msq = p1.tile([8, 2], F32, name="msq", tag=f"msq{i}")
                    nc.scalar.activation(out=msq[:, 0:1], in_=stat2[:, 0:1], func=ACTF.Square)
                    nc.vector.tensor_add(out=msq[:, 1:2], in0=z_sb[:, 1:2], in1=z_sb[:, 2:3])
                    # var = (z1+z2)/16 - mean^2 ; rstd = 1/sqrt(var+eps)
                    nc.vector.scalar_tensor_tensor(
                        out=msq[:, 1:2], in0=msq[:, 1:2], scalar=1.0 / GSIZE,
                        in1=msq[:, 0:1], op0=ALU.mult, op1=ALU.subtract,
                    )
                    nc.scalar.activation(out=msq[:, 1:2], in_=msq[:, 1:2], func=ACTF.Sqrt, bias=eps_sb)
                    nc.vector.reciprocal(out=stat2[:, 1:2], in_=msq[:, 1:2])
                    mr = p1p.tile([128, 2], F32, name="mr", tag="mr", bufs=2)
                    nc.tensor.matmul(mr, ind_sb, stat2, start=True, stop=True)
                    # scale = gamma * rstd ; shift = beta - mean * scale
                    nc.vector.tensor_mul(
                        out=scale_sb[i], in0=gam_sb[:, i : i + 1], in1=mr[:, 1:2]
                    )
                    tmp_sh = p1.tile([128, 1], F32, name="tmp_sh", tag=f"tmp_sh{i}")
                    nc.vector.tensor_scalar_mul(out=tmp_sh, in0=mr[:, 0:1], scalar1=scale_sb[i])
                    nc.vector.tensor_sub(out=shift_sb[i], in0=bet_sb[:, i : i + 1], in1=tmp_sh)

                st2s = {0: bn_chunk(0), 1: bn_chunk(1)}
                chain_chunk(0, st2s[0])
                st2s[2] = bn_chunk(2)
                chain_chunk(1, st2s[1])
                st2s[3] = bn_chunk(3)
                chain_chunk(2, st2s[2])
                chain_chunk(3, st2s[3])

                # projections. k/v first (phase 2's scores need all keys),
                # then q per query window; hn/hm production on gpsimd (the
                # only engine with slack; SBUF-only so Pool is allowed).
                for nw in range(NW):
                    nsl = bass.ts(nw, 512)
                    hn2 = [
                        p1.tile([128, 2, 512], FP8, name="hn", tag="hn", bufs=4)
                        for _ in range(2)
                    ]
                    hn = {0: hn2[0][:, 0, :], 1: hn2[0][:, 1, :],
                          2: hn2[1][:, 0, :], 3: hn2[1][:, 1, :]}
                    for c in range(NCH):
                        nc.vector.tensor_scalar(
                            out=hn[c], in0=xf[c][:, nsl],
                            scalar1=scale_sb[c], scalar2=shift_sb[c],
                            op0=ALU.mult, op1=ALU.add,
                        )
                    for co in range(NCH):
                        pq = p1p.tile([128, 512], F32, name="pq", tag="pq", bufs=2)
                        for ks in range(NCH // 2):
                            nc.tensor.matmul(
                                pq,
                                wq_sb[:, 2 * ks : 2 * ks + 2, bass.ts(co, 128)],
                                hn2[ks],
                                start=(ks == 0), stop=(ks == NCH // 2 - 1),
                                perf_mode=DR,
                            )
                        nc.scalar.activation(
                            out=q_sb[:, co, nsl], in_=pq, func=ACTF.Identity,
                            bias=bq_sb[:, co : co + 1], scale=1.0 / WS,
                        )
                for mw in range((nkm + 511) // 512):
                    lo = mw * 512
                    wsz = min(512, nkm - lo)
                    msl = slice(lo, lo + wsz)
                    hm2 = [
                        p1.tile([128, 2, 512], FP8, name="hm", tag="hn", bufs=4)
                        for _ in range(2)
                    ]
                    hm = {0: hm2[0][:, 0, :], 1: hm2[0][:, 1, :],
                          2: hm2[1][:, 0, :], 3: hm2[1][:, 1, :]}
                    for c in range(NCH):
                        nc.scalar.activation(
                            out=hm[c][:, :wsz], in_=xm_sb[c][:, msl],
                            func=ACTF.Identity,
                            scale=scale_sb[c], bias=shift_sb[c],
                        )
                    for co in range(NCH):
                        pk = p1p.tile([128, 512], F32, name="pk", tag="pk", bufs=2)
                        for ks in range(NCH // 2):
                            nc.tensor.matmul(
                                pk[:, :wsz],
                                wk_sb[:, 2 * ks : 2 * ks + 2, bass.ts(co, 128)],
                                hm2[ks][:, :, :wsz],
                                start=(ks == 0), stop=(ks == NCH // 2 - 1),
                                perf_mode=DR,
                            )
                        nc.scalar.activation(
                            out=k_sb[:, co, msl], in_=pk[:, :wsz], func=ACTF.Identity,
                            bias=bk_sb[:, co : co + 1], scale=1.0 / WS,
                        )
                    # v, produced transposed: vt[key, c_out] = hm[c_in,key].T @ wvt
                    for kw in range(wsz // 128):
                        pv = p1p.tile([128, C], F32, name="pv", tag="pv", bufs=2)
                        for ks in range(NCH // 2):
                            nc.tensor.matmul(
                                pv,
                                hm2[ks][:, :, bass.ts(kw, 128)],
                                wv_sb[:, 2 * ks : 2 * ks + 2, :],
                                start=(ks == 0), stop=(ks == NCH // 2 - 1),
                                perf_mode=DR,
                            )
                        nc.vector.tensor_scalar_mul(
                            out=vt_sb[:, mw * 4 + kw, :], in0=pv, scalar1=1.0 / WS
                        )

            if loop_phase == "ph1":
                loop_ctx.close()
            elif loop_phase == "ph2":
                if loop_n > 1:
                    loop_ctx.enter_context(tc.For_i(0, loop_n, 1))

            # ================= phase 2: attention =================
            with (
                tc.tile_pool(name="ph2", bufs=1) as p2,
                tc.tile_pool(name="ph2psum", bufs=1, space="PSUM") as p2p,
            ):
                NWP = NKWM // 2       # full window pairs (DoubleRow)
                leftover = NKWM % 2   # odd tail window -> plain fp8 matmuls
                for qt in range(NQT):
                    qsl = bass.ts(qt, QT)
                    out_ps = [
                        p2p.tile([128, QT], F32, name="out_ps", tag="out", bufs=4)
                        for _ in range(NCH)
                    ]
                    acc = [
                        p2.tile([128, QT], BF16, name="dacc", tag="dacc", bufs=4)
                        for _ in range(2)
                    ]

                    def scores(w, pt_sl):
                        sc = p2p.tile([128, QT], F32, name="sc", tag="sc", bufs=3)
                        for ks in range(NCH // 2):
                            nc.tensor.matmul(
                                sc,
                                k_sb[:, 2 * ks : 2 * ks + 2, bass.ts(w, 128)],
                                q_sb[:, 2 * ks : 2 * ks + 2, qsl],
                                start=(ks == 0), stop=(ks == NCH // 2 - 1),
                                perf_mode=DR,
                            )
                        # p = exp(s/sqrt(C) + logmask_k - SHIFT)
                        nc.scalar.activation(
                            out=pt_sl, in_=sc, func=ACTF.Exp,
                            bias=lm_sb[:, w : w + 1], scale=qscale,
                        )

                    for wp in range(NWP):
                        pt = p2.tile([128, 2, QT], FP8, name="pt", tag="pt", bufs=6)
                        for h in range(2):
                            scores(2 * wp + h, pt[:, h, :])
                        for c in range(NCH):
                            nc.tensor.matmul(
                                out_ps[c],
                                vt_sb[:, 2 * wp : 2 * wp + 2, bass.ts(c, 128)],
                                pt,
                                start=(wp == 0),
                                stop=(wp == NWP - 1 and not leftover),
                                perf_mode=DR,
                            )
                        # denominator on DVE: two split accumulators to
                        # shorten the read-modify-write chain
                        if wp < 2:
                            nc.vector.tensor_add(
                                out=acc[wp], in0=pt[:, 0, :], in1=pt[:, 1, :]
                            )
                        else:
                            dtmp = p2.tile([128, QT], BF16, name="dtmp", tag="dtmp", bufs=3)
                            nc.vector.tensor_add(
                                out=dtmp, in0=pt[:, 0, :], in1=pt[:, 1, :]
                            )
                            nc.vector.tensor_add(
                                out=acc[wp % 2], in0=acc[wp % 2], in1=dtmp
                            )
                    if leftover:
                        w = NKWM - 1
                        pts = p2.tile([128, QT], FP8, name="pts", tag="pts", bufs=2)
                        scores(w, pts)
                        for c in range(NCH):
                            nc.tensor.matmul(
                                out_ps[c], vt_sb[:, w, bass.ts(c, 128)], pts,
                                start=False, stop=True,
                            )
                        nc.vector.tensor_add(out=acc[1], in0=acc[1], in1=pts)
                    nc.vector.tensor_add(out=acc[0], in0=acc[0], in1=acc[1])
                    ds_ps = p2p.tile([128, QT], F32, name="ds_ps", tag="ds", bufs=1)
                    nc.tensor.matmul(ds_ps, ones_sb, acc[0], start=True, stop=True)
                    dinv = p2.tile([128, QT], F32, name="dinv", tag="dinv", bufs=2)
                    nc.vector.reciprocal(out=dinv, in_=ds_ps)
                    # outn = (out_ps * WS) / denominator; x16 rides into fp8
                    o4 = p2.tile([128, NCH, QT], FP8, name="o4", tag="o4", bufs=2)
                    for c in range(NCH):
                        nc.vector.scalar_tensor_tensor(
                            out=o4[:, c, :], in0=out_ps[c], scalar=WS,
                            in1=dinv, op0=ALU.mult, op1=ALU.mult,
                        )
                    for co in range(NCH):
                        pj = p2p.tile([128, QT], F32, name="pj", tag="out", bufs=4)
                        for ks in range(NCH // 2):
                            nc.tensor.matmul(
                                pj,
                                wo_sb[:, 2 * ks : 2 * ks + 2, bass.ts(co, 128)],
                                o4[:, 2 * ks : 2 * ks + 2, :],
                                start=(ks == 0), stop=(ks == NCH // 2 - 1),
                                perf_mode=DR,
                            )
                        xres = p2.tile([128, QT], F32, name="xres", tag="xres", bufs=3)
                        nc.sync.dma_start(
                            out=xres, in_=xq_d[bass.ts(co, 128), qsl]
                        )
                        # y = (x + bo2) + pj/(WS*WS)   (bo2 folded into xq host-side)
                        ym = p2.tile([128, QT], F32, name="ym", tag="ym", bufs=3)
                        nc.scalar.activation(
                            out=ym, in_=pj, func=ACTF.Identity, scale=1.0 / (WS * WS),
                        )
                        y_t = p2.tile([128, QT], F32, name="y_t", tag="yt", bufs=3)
                        nc.vector.tensor_add(out=y_t, in0=ym, in1=xres)
                        nc.sync.dma_start(out=y_d[bass.ts(co, 128), qsl], in_=y_t)

            loop_ctx.close()

    nc.finalize()
    return nc


_prog_cache = {}


def _get_program(loop_n: int = 1, loop_phase: str = "all", nkm: int = NKM):
    key = (loop_n, loop_phase, nkm)
    if key not in _prog_cache:
        _prog_cache[key] = build_program(loop_n, loop_phase, nkm)
    return _prog_cache[key]


def _prep_in_maps(x, mask, gamma, beta, Wq, bq, Wk, bk, Wv, bv, Wo, bo, nkm=NKM):
    x = np.asarray(x, np.float32).reshape(B, C, HW)
    mask = np.asarray(mask, np.float32)
    bf = ml_dtypes.bfloat16
    f8 = ml_dtypes.float8_e4m3

    def wprep(W):
        # [out, in] -> transpose -> scale x16 -> [128, NCH, C] fp8
        wt = np.asarray(W, np.float32).T * WS
        wt = np.clip(wt, -240.0, 240.0)
        return np.ascontiguousarray(
            wt.reshape(NCH, 128, C).transpose(1, 0, 2)
        ).astype(f8)

    bo2 = (np.asarray(Wo, np.float32) @ np.asarray(bv, np.float32)
           + np.asarray(bo, np.float32)).astype(np.float32)
    shared = {
        "gamma": np.asarray(gamma, np.float32),
        "beta": np.asarray(beta, np.float32),
        "wqd": wprep(Wq),
        "wkd": wprep(Wk),
        "wvd": wprep(Wv),
        "wod": wprep(Wo),
        "bq": np.asarray(bq, np.float32),
        "bk": np.asarray(bk, np.float32),
        "ind": (np.arange(128)[None, :] // GSIZE == np.arange(8)[:, None]).astype(
            np.float32
        ),
        "ind2": (np.arange(128)[:, None] // GSIZE == np.arange(8)[None, :]).astype(
            np.float32
        ),
    }
    in_maps = []
    for core in range(8):
        b, half = core // 2, core % 2
        xb, mb = x[b], mask[b]
        if half == 1:
            xb = np.concatenate([xb[:, NQ:], xb[:, :NQ]], axis=1)
            mb = np.concatenate([mb[NQ:], mb[:NQ]])
        # compact the keys: only masked-in columns take part in attention
        idx = np.nonzero(mb > 0.5)[0]
        nk = len(idx)
        assert nk <= nkm, f"mask density too high: {nk} > {nkm}"
        xm = np.zeros((C, nkm), dtype=bf)
        xm[:, :nk] = xb[:, idx].astype(bf)
        lm = np.full(nkm, NEG, np.float32)
        lm[:nk] = -SHIFT
        in_maps.append(
            {
                "xbf": xb.astype(bf),
                # residual input with the folded output bias pre-added
                "xq": np.ascontiguousarray(xb[:, :NQ]) + bo2[:, None],
                "xm": xm,
                "lmask": lm,
                **shared,
            }
        )
    return in_maps


def _pick_nkm(mask):
    mask = np.asarray(mask, np.float32)
    max_nk = int((mask > 0.5).sum(axis=1).max())
    return max(NKM, ((max_nk + 255) // 256) * 256)


def kernel(x, mask, gamma, beta, Wq, bq, Wk, bk, Wv, bv, Wo, bo):
    nkm = _pick_nkm(mask)
    nc = _get_program(nkm=nkm)
    in_maps = _prep_in_maps(
        x, mask, gamma, beta, Wq, bq, Wk, bk, Wv, bv, Wo, bo, nkm=nkm
    )
    res = run_bass_kernel_spmd(nc, in_maps, list(range(8)))
    out = np.empty((B, C, HW), np.float32)
    for core in range(8):
        b, half = core // 2, core % 2
        out[b, :, half * NQ : (half + 1) * NQ] = res.results[core]["y"]
    return out.reshape(B, C, HGT, WID)
